# revision 10
# baseline (speedup 1.0000x reference)
"""GQA attention (B=2, T=2048, D=2048, 32 heads / 8 KV groups, head_dim=64,
RoPE, causal) distributed over 8 TRN2 NeuronCores.

Sharding: core i handles batch b = i//4 and KV-group pair (2*(i%4), 2*(i%4)+1),
i.e. 8 query heads + 2 KV heads. QKV is column-sharded, out-proj row-sharded;
each core writes a partial [T, D] output (bf16) and the host sums 4 partials
per batch. No collectives.

v2 design notes (vs the 555us baseline):
 - every matmul is 128x128-tile-mode: scores use a zero-padded K=128 moving
   operand (q for the group-pair stacked with explicit zeros) against the
   2-group stacked K stationary, so the PE never switches tiling mode (the
   baseline's 64-row scores forced a pipeline drain per matmul and kept the
   HAM clock throttled at 1.2 GHz for the whole attention phase)
 - softmax exp is one ACT instruction per kt-tile covering BOTH heads via a
   [128, 2, w] access pattern over a two-bank PSUM tile (halves ACT overhead)
 - QKV / attention / out-proj are chunk-pipelined (512-token chunks) so the
   scalar/vector/DMA work of one stage hides under the PE work of another
 - all PSUM evacuations ride the vector engine; output partials are bf16
"""

import sys

sys.path.insert(0, "/opt/trn_rl_repo")

from contextlib import ExitStack

import numpy as np
import ml_dtypes

from concourse import bacc, mybir, tile
from concourse.bass_utils import run_bass_kernel_spmd

# problem constants (hardcoded per contract)
B, T, D = 2, 2048, 2048
N_HEAD, N_GROUPS, HEAD_DIM = 32, 8, 64
KV_DIM = N_GROUPS * HEAD_DIM  # 512
NCORES = 8
WCOLS = 768  # 512 q + 128 k + 128 v per core

F32 = mybir.dt.float32
BF16 = mybir.dt.bfloat16
TQ = 512  # token chunk
NT = T // TQ  # 4
NCT = D // 128  # 16 contraction tiles for QKV
SCALE = float(HEAD_DIM) ** -0.5


# ---------------------------------------------------------------- host tables


def _host_tables():
    theta = 1.0 / (10000.0 ** (np.arange(0, HEAD_DIM, 2, dtype=np.float64) / HEAD_DIM))
    freqs = np.arange(T, dtype=np.float64)[None, :] * theta[:, None]  # [32, T]
    cos64 = np.repeat(np.cos(freqs), 2, axis=0)  # rows 2i,2i+1 -> cos_i
    sin64 = np.repeat(np.sin(freqs), 2, axis=0)
    sgn = np.where(np.arange(HEAD_DIM) % 2 == 0, -1.0, 1.0)[:, None]
    cos128 = np.concatenate([cos64, cos64], 0).astype(np.float32)  # [128, T]
    sin128 = np.concatenate([sin64 * sgn, sin64 * sgn], 0).astype(np.float32)

    swp = np.zeros((128, 128), np.float32)  # swap(q)[d] = q[d^1]
    for d in range(128):
        swp[d ^ 1, d] = 1.0

    kt = np.arange(128)[:, None]
    qt = np.arange(128)[None, :]
    umask = (qt >= kt).astype(np.float32)  # [kt, qt] causal keep-mask
    umask2 = np.stack([umask, umask], axis=1)  # [128, 2, 128] (both heads)

    selb = np.zeros((128, 128), np.float32)  # va_g[kt,d] = v_sb[64g+d, kt]
    for d in range(64):
        selb[d, d] = 1.0  # cols 0-63: group 0
        selb[64 + d, 64 + d] = 1.0  # cols 64-127: group 1

    # sel4[jj]: bcast rows 2jj / 2jj+1 -> psum rows 0-63 / 64-127
    sel4 = np.zeros((4, 128, 128), np.float32)
    for jj in range(4):
        sel4[jj, 2 * jj, :64] = 1.0
        sel4[jj, 2 * jj + 1, 64:] = 1.0
    bf = ml_dtypes.bfloat16
    return (cos128, sin128, swp.astype(bf), umask2.astype(bf),
            selb.astype(bf), sel4.astype(bf))


def _shard_inputs(x, w_qkv, w_proj):
    """Per-core input dicts. Core i: batch i//4, group pair gp = i%4."""
    cos128, sin128, swp, umask2, selb, sel4 = _host_tables()
    xt = [np.ascontiguousarray(x[b].T.astype(ml_dtypes.bfloat16)) for b in range(B)]  # [D, T]
    maps = []
    for i in range(NCORES):
        b, gp = i // 4, i % 4
        heads = [8 * gp + j for j in range(8)]  # global heads of this core
        # q blocks pair local heads (j, j+4) = (group 2gp head j, group 2gp+1 head j)
        qcols = []
        for j in range(4):
            qcols.append(w_qkv[:, 64 * heads[j] : 64 * heads[j] + 64])
            qcols.append(w_qkv[:, 64 * heads[j + 4] : 64 * heads[j + 4] + 64])
        kcol = w_qkv[:, D + 128 * gp : D + 128 * gp + 128]
        vcol = w_qkv[:, D + KV_DIM + 128 * gp : D + KV_DIM + 128 * gp + 128]
        wq = np.ascontiguousarray(np.concatenate(qcols + [kcol, vcol], axis=1).astype(ml_dtypes.bfloat16))
        # w_proj rows in ypair order: pair j = [head j ; head j+4]
        wrows = []
        for j in range(4):
            wrows.append(w_proj[64 * heads[j] : 64 * heads[j] + 64, :])
            wrows.append(w_proj[64 * heads[j + 4] : 64 * heads[j + 4] + 64, :])
        wp = np.ascontiguousarray(np.concatenate(wrows, axis=0).astype(ml_dtypes.bfloat16))
        maps.append(
            {
                "xt": xt[b],
                "wqkv": wq,
                "wproj": wp,
                "costab": cos128,
                "sintab": sin128,
                "swp": swp,
                "umask2": umask2,
                "selb": selb,
                "sel4": sel4,
            }
        )
    return maps


# ------------------------------------------------------------------- builder


def build_nc():
    nc = bacc.Bacc("TRN2", target_bir_lowering=False, debug=False, num_devices=NCORES)
    xt_d = nc.dram_tensor("xt", [D, T], BF16, kind="ExternalInput").ap()
    wq_d = nc.dram_tensor("wqkv", [D, WCOLS], BF16, kind="ExternalInput").ap()
    wp_d = nc.dram_tensor("wproj", [512, D], BF16, kind="ExternalInput").ap()
    cos_d = nc.dram_tensor("costab", [128, T], F32, kind="ExternalInput").ap()
    sin_d = nc.dram_tensor("sintab", [128, T], F32, kind="ExternalInput").ap()
    swp_d = nc.dram_tensor("swp", [128, 128], BF16, kind="ExternalInput").ap()
    um2_d = nc.dram_tensor("umask2", [128, 2, 128], BF16, kind="ExternalInput").ap()
    slb_d = nc.dram_tensor("selb", [128, 128], BF16, kind="ExternalInput").ap()
    s4_d = nc.dram_tensor("sel4", [4, 128, 128], BF16, kind="ExternalInput").ap()
    out_d = nc.dram_tensor("out", [T, D], BF16, kind="ExternalOutput").ap()

    with (
        nc.allow_low_precision(reason="bf16 matmul operands; fp32 psum accumulation"),
        tile.TileContext(nc) as tc,
        ExitStack() as ctx,
    ):
        const = ctx.enter_context(tc.tile_pool(name="const", bufs=1))
        keep = ctx.enter_context(tc.tile_pool(name="keep", bufs=1))
        p_x = ctx.enter_context(tc.tile_pool(name="p_x", bufs=32))
        p_w = ctx.enter_context(tc.tile_pool(name="p_w", bufs=2))
        p_yh = ctx.enter_context(tc.tile_pool(name="p_yh", bufs=4))
        p_pt = ctx.enter_context(tc.tile_pool(name="p_pt", bufs=3))
        ps_a = ctx.enter_context(tc.tile_pool(name="ps_a", bufs=1, space="PSUM"))
        ps_sc = ctx.enter_context(tc.tile_pool(name="ps_sc", bufs=2, space="PSUM"))
        ps_pv = ctx.enter_context(tc.tile_pool(name="ps_pv", bufs=1, space="PSUM"))

        cos_t = const.tile([128, T], F32)
        sin_t = const.tile([128, T], F32)
        swp_t = const.tile([128, 128], BF16)
        um2_t = const.tile([128, 2, 128], BF16)
        slb_t = const.tile([128, 128], BF16)
        s4_t = const.tile([128, 4, 128], BF16)
        wq_sb = [
            keep.tile([128, WCOLS], BF16, tag=f"wq{ci}", name=f"wq{ci}")
            for ci in range(NCT)
        ]
        wp_sb = keep.tile([128, 4, D], BF16, tag="wp", name="wp_sb")

        # persistent per-chunk activations
        qpad = [
            [
                [
                    keep.tile([128, TQ], BF16, tag=f"qp{c}_{jp}_{h}", name=f"qp{c}_{jp}_{h}")
                    for h in range(2)
                ]
                for jp in range(4)
            ]
            for c in range(NT)
        ]
        kp_c = [keep.tile([128, TQ], BF16, tag=f"kp{c}", name=f"kp{c}") for c in range(NT)]
        v_c = [keep.tile([128, TQ], BF16, tag=f"v{c}", name=f"v{c}") for c in range(NT)]
        va_c = [
            [keep.tile([128, 4, 65], BF16, tag=f"va{c}_{g}", name=f"va{c}_{g}") for g in range(2)]
            for c in range(NT)
        ]
        yp_c = [
            [keep.tile([128, TQ], BF16, tag=f"yp{c}_{jj}", name=f"yp{c}_{jj}") for jj in range(4)]
            for c in range(NT)
        ]
        sump_c = [keep.tile([128, TQ], BF16, tag=f"sump{c}", name=f"sump{c}") for c in range(NT)]

        # sump rows 8-127 are never written but are read by the bcast matmul
        # (times zero stationary cols) and the denominator reciprocal -- they
        # must be finite; qpad padding halves must be exact zeros for the
        # K=128 padded scores
        for c in range(NT):
            nc.gpsimd.memset(sump_c[c][:], 1.0)
        for c in range(NT):
            for jp in range(4):
                nc.gpsimd.memset(qpad[c][jp][0][64:128, :], 0.0)
                nc.gpsimd.memset(qpad[c][jp][1][0:64, :], 0.0)

        xts = [[None] * NCT for _ in range(NT)]

        def dma_consts():
            for t_, d_ in [
                (um2_t, um2_d),
                (slb_t, slb_d),
            ]:
                nc.sync.dma_start(t_[:], d_)
            for jj in range(4):
                nc.sync.dma_start(s4_t[:, jj, :], s4_d[jj, :, :])
            for fi in range(4):
                nc.sync.dma_start(wp_sb[:, fi, :], wp_d[128 * fi : 128 * fi + 128, :])

        def dma_xt(c):
            ts = slice(TQ * c, TQ * c + TQ)
            for ci in range(NCT):
                xt_t = p_x.tile([128, TQ], BF16, tag="xt", name=f"xt{c}_{ci}")
                nc.sync.dma_start(xt_t[:], xt_d[128 * ci : 128 * ci + 128, ts])
                xts[c][ci] = xt_t

        def rope(c, ps, dest_q=None, dest_k=None):
            ts = slice(TQ * c, TQ * c + TQ)
            raw = p_w.tile([128, TQ], BF16, tag="raw", name="raw")
            nc.vector.tensor_copy(raw[:], ps[:])
            sw = ps_a.tile([128, TQ], F32, tag="qkv", name="sw")
            nc.tensor.matmul(sw[:], swp_t[:], raw[:], start=True, stop=True)
            t1 = p_w.tile([128, TQ], F32, tag="t1", name="t1")
            t2 = p_w.tile([128, TQ], F32, tag="t2", name="t2")
            nc.vector.tensor_mul(t1[:], raw[:], cos_t[:, ts])
            nc.vector.tensor_mul(t2[:], sw[:], sin_t[:, ts])
            if dest_q is not None:
                nc.vector.tensor_add(dest_q[0][0:64, :], t1[0:64, :], t2[0:64, :])
                nc.vector.tensor_add(dest_q[1][64:128, :], t1[64:128, :], t2[64:128, :])
            else:
                nc.vector.tensor_add(dest_k[:], t1[:], t2[:])

        def qkv_part(c, ocs):
            for oc in ocs:
                ps = ps_a.tile([128, TQ], F32, tag="qkv", name="qkv")
                for ci in range(NCT):
                    nc.tensor.matmul(
                        ps[:],
                        wq_sb[ci][:, 128 * oc : 128 * oc + 128],
                        xts[c][ci][:],
                        start=(ci == 0),
                        stop=(ci == NCT - 1),
                    )
                if oc < 4:
                    rope(c, ps, dest_q=qpad[c][oc])
                elif oc == 4:
                    rope(c, ps, dest_k=kp_c[c])
                else:
                    nc.vector.tensor_copy(v_c[c][:], ps[:])
                    for kloc in range(4):
                        vp = ps_a.tile([128, 128], F32, tag="qkv", name="vp")
                        nc.tensor.matmul(
                            vp[:],
                            v_c[c][:, 128 * kloc : 128 * kloc + 128],
                            slb_t[:],
                            start=True,
                            stop=True,
                        )
                        nc.vector.tensor_copy(va_c[c][0][:, kloc, 0:64], vp[:, 0:64])
                        nc.vector.tensor_copy(va_c[c][1][:, kloc, 0:64], vp[:, 64:128])
                    for g in range(2):
                        nc.gpsimd.memset(va_c[c][g][:, :, 64:65], 1.0)

        def attn(s, jp):
            pv = [
                ps_pv.tile([65, TQ], F32, tag=f"pv{h}", name=f"pv{h}") for h in range(2)
            ]
            nkj = 4 * s + 4
            for kj in range(nkj):
                qcs, kloc = kj // 4, kj % 4
                col0 = max(kj * 128 - s * TQ, 0)
                sc = ps_sc.tile([128, 2, TQ], F32, tag="sc", name="sc")
                for h in range(2):
                    nc.tensor.matmul(
                        sc[:, h, col0:TQ],
                        kp_c[qcs][:, 128 * kloc : 128 * kloc + 128],
                        qpad[s][jp][h][:, col0:TQ],
                        start=True,
                        stop=True,
                    )
                pt = p_pt.tile([128, 2, TQ], BF16, tag="pt", name="pt")
                nc.scalar.activation(
                    pt[:, :, col0:TQ],
                    sc[:, :, col0:TQ],
                    mybir.ActivationFunctionType.Exp,
                    scale=SCALE,
                )
                if kj >= 4 * s:  # diagonal tile: triangular keep-mask, both heads
                    nc.gpsimd.tensor_mul(
                        pt[:, :, col0 : col0 + 128],
                        pt[:, :, col0 : col0 + 128],
                        um2_t[:],
                    )
                for h in range(2):
                    nc.tensor.matmul(
                        pv[h][:, col0:TQ],
                        va_c[qcs][h][:, kloc, :],
                        pt[:, h, col0:TQ],
                        start=(kj == 0),
                        stop=(kj == nkj - 1),
                    )
            nc.vector.tensor_copy(yp_c[s][jp][0:64, :], pv[0][0:64, :])
            dn = p_yh.tile([65, TQ], BF16, tag="dn", name="dn")
            nc.vector.tensor_copy(dn[64:65, :], pv[0][64:65, :])
            nc.sync.dma_start(sump_c[s][2 * jp : 2 * jp + 1, :], dn[64:65, :])
            yh = p_yh.tile([65, TQ], BF16, tag="yh", name="yh")
            nc.vector.tensor_copy(yh[:], pv[1][:])
            nc.sync.dma_start(yp_c[s][jp][64:128, :], yh[0:64, :])
            nc.sync.dma_start(sump_c[s][2 * jp + 1 : 2 * jp + 2, :], yh[64:65, :])

        def norm(c):
            nc.vector.reciprocal(sump_c[c][0:8, :], sump_c[c][0:8, :])
            for jj in range(4):
                bc = ps_a.tile([128, TQ], F32, tag="pj", name="bc")
                nc.tensor.matmul(
                    bc[:], s4_t[:, jj, :], sump_c[c][:], start=True, stop=True
                )
                nc.vector.tensor_mul(yp_c[c][jj][:], yp_c[c][jj][:], bc[:])

        def proj_part(c, tb):
            tok0 = c * TQ + tb * 128
            for oc in range(4):
                pj = ps_a.tile(
                    [128, TQ], F32, tag="pj" if (tb * 4 + oc) % 2 == 0 else "qkv",
                    name="pj",
                )
                for jj in range(4):
                    nc.tensor.matmul(
                        pj[:],
                        yp_c[c][jj][:, 128 * tb : 128 * tb + 128],
                        wp_sb[:, jj, TQ * oc : TQ * oc + TQ],
                        start=(jj == 0),
                        stop=(jj == 3),
                    )
                ot = p_w.tile([128, TQ], BF16, tag="ot", name="ot")
                nc.vector.tensor_copy(ot[:], pj[:])
                nc.sync.dma_start(out_d[tok0 : tok0 + 128, TQ * oc : TQ * oc + TQ], ot[:])

        # ---------------- emission: chunk-pipelined schedule
        # qkv-critical loads paired per-ci so the first matmul starts ~2us in
        for ci in range(NCT):
            nc.sync.dma_start(wq_sb[ci][:], wq_d[128 * ci : 128 * ci + 128, :])
            xt_t = p_x.tile([128, TQ], BF16, tag="xt", name=f"xt0_{ci}")
            nc.sync.dma_start(xt_t[:], xt_d[128 * ci : 128 * ci + 128, 0:TQ])
            xts[0][ci] = xt_t
            if ci == 7:
                nc.sync.dma_start(cos_t[:], cos_d)
                nc.sync.dma_start(sin_t[:], sin_d)
                nc.sync.dma_start(swp_t[:], swp_d)
        dma_consts()
        dma_xt(1)
        qkv_part(0, [4, 5])
        oc_slices = [[0, 1], [2, 3], [4], [5]]
        for s in range(NT):
            if s >= 1 and s + 1 < NT:
                dma_xt(s + 1)  # chunk s+1 inputs, consumed by qkv_part this step
            for jp in range(4):
                if s == 0:
                    qkv_part(0, [jp])
                attn(s, jp)
                if jp == 0 and s >= 1:
                    norm(s - 1)  # bcast matmuls queue behind attn: recip hides
                if s < NT - 1:
                    qkv_part(s + 1, oc_slices[jp])
                if s >= 1:
                    proj_part(s - 1, jp)
        norm(NT - 1)
        for tb in range(4):
            proj_part(NT - 1, tb)

    nc.compile()
    return nc


_NC_CACHE = None


def _get_nc():
    global _NC_CACHE
    if _NC_CACHE is None:
        _NC_CACHE = build_nc()
    return _NC_CACHE


def kernel(x, w_qkv, w_proj, _trace=False, _nc=None):
    x = np.asarray(x, np.float32)
    w_qkv = np.asarray(w_qkv, np.float32)
    w_proj = np.asarray(w_proj, np.float32)
    nc = _nc if _nc is not None else _get_nc()
    in_maps = _shard_inputs(x, w_qkv, w_proj)
    res = run_bass_kernel_spmd(nc, in_maps, core_ids=list(range(NCORES)), trace=_trace)
    out = np.zeros((B, T, D), np.float32)
    for i in range(NCORES):
        out[i // 4] += res.results[i]["out"].astype(np.float32)
    if _trace:
        return out, res
    return out


if __name__ == "__main__":
    rng = np.random.default_rng(0)
    x = rng.standard_normal((B, T, D), dtype=np.float32)
    wq = rng.standard_normal((D, D + 2 * KV_DIM), dtype=np.float32) * D**-0.5
    wp = rng.standard_normal((D, D), dtype=np.float32) * D**-0.5
    y = kernel(x, wq, wp)
    print(y.shape, y.dtype)


# revision 11
# speedup vs baseline: 1.0753x; 1.0753x over previous
"""GQA attention (B=2, T=2048, D=2048, 32 heads / 8 KV groups, head_dim=64,
RoPE, causal) distributed over 8 TRN2 NeuronCores.

Sharding: core i handles batch b = i//4 and KV-group pair (2*(i%4), 2*(i%4)+1),
i.e. 8 query heads + 2 KV heads. QKV is column-sharded, out-proj row-sharded;
each core writes a partial [T, D] output (bf16) and the host sums 4 partials
per batch. No collectives.

v2 design notes (vs the 555us baseline):
 - every matmul is 128x128-tile-mode: scores use a zero-padded K=128 moving
   operand (q for the group-pair stacked with explicit zeros) against the
   2-group stacked K stationary, so the PE never switches tiling mode (the
   baseline's 64-row scores forced a pipeline drain per matmul and kept the
   HAM clock throttled at 1.2 GHz for the whole attention phase)
 - softmax exp is one ACT instruction per kt-tile covering BOTH heads via a
   [128, 2, w] access pattern over a two-bank PSUM tile (halves ACT overhead)
 - QKV / attention / out-proj are chunk-pipelined (512-token chunks) so the
   scalar/vector/DMA work of one stage hides under the PE work of another
 - all PSUM evacuations ride the vector engine; output partials are bf16
"""

import sys

sys.path.insert(0, "/opt/trn_rl_repo")

from contextlib import ExitStack

import numpy as np
import ml_dtypes

from concourse import bacc, mybir, tile
from concourse.bass_utils import run_bass_kernel_spmd

# problem constants (hardcoded per contract)
B, T, D = 2, 2048, 2048
N_HEAD, N_GROUPS, HEAD_DIM = 32, 8, 64
KV_DIM = N_GROUPS * HEAD_DIM  # 512
NCORES = 8
WCOLS = 768  # 512 q + 128 k + 128 v per core

F32 = mybir.dt.float32
BF16 = mybir.dt.bfloat16
TQ = 512  # token chunk
NT = T // TQ  # 4
NCT = D // 128  # 16 contraction tiles for QKV
SCALE = float(HEAD_DIM) ** -0.5


# ---------------------------------------------------------------- host tables


def _host_tables():
    theta = 1.0 / (10000.0 ** (np.arange(0, HEAD_DIM, 2, dtype=np.float64) / HEAD_DIM))
    freqs = np.arange(T, dtype=np.float64)[None, :] * theta[:, None]  # [32, T]
    cos64 = np.repeat(np.cos(freqs), 2, axis=0)  # rows 2i,2i+1 -> cos_i
    sin64 = np.repeat(np.sin(freqs), 2, axis=0)
    sgn = np.where(np.arange(HEAD_DIM) % 2 == 0, -1.0, 1.0)[:, None]
    cos128 = np.concatenate([cos64, cos64], 0).astype(np.float32)  # [128, T]
    sin128 = np.concatenate([sin64 * sgn, sin64 * sgn], 0).astype(np.float32)

    swp = np.zeros((128, 128), np.float32)  # swap(q)[d] = q[d^1]
    for d in range(128):
        swp[d ^ 1, d] = 1.0

    kt = np.arange(128)[:, None]
    qt = np.arange(128)[None, :]
    umask = (qt >= kt).astype(np.float32)  # [kt, qt] causal keep-mask
    umask2 = np.stack([umask, umask], axis=1)  # [128, 2, 128] (both heads)

    selb = np.zeros((128, 128), np.float32)  # va_g[kt,d] = v_sb[64g+d, kt]
    for d in range(64):
        selb[d, d] = 1.0  # cols 0-63: group 0
        selb[64 + d, 64 + d] = 1.0  # cols 64-127: group 1

    # sel4[jj]: bcast rows 2jj / 2jj+1 -> psum rows 0-63 / 64-127
    sel4 = np.zeros((4, 128, 128), np.float32)
    for jj in range(4):
        sel4[jj, 2 * jj, :64] = 1.0
        sel4[jj, 2 * jj + 1, 64:] = 1.0
    bf = ml_dtypes.bfloat16
    return (cos128, sin128, swp.astype(bf), umask2.astype(bf),
            selb.astype(bf), sel4.astype(bf))


def _shard_inputs(x, w_qkv, w_proj):
    """Per-core input dicts. Core i: batch i//4, group pair gp = i%4."""
    cos128, sin128, swp, umask2, selb, sel4 = _host_tables()
    xt = [np.ascontiguousarray(x[b].T.astype(ml_dtypes.bfloat16)) for b in range(B)]  # [D, T]
    maps = []
    for i in range(NCORES):
        b, gp = i // 4, i % 4
        heads = [8 * gp + j for j in range(8)]  # global heads of this core
        # q blocks pair local heads (j, j+4) = (group 2gp head j, group 2gp+1 head j)
        qcols = []
        for j in range(4):
            qcols.append(w_qkv[:, 64 * heads[j] : 64 * heads[j] + 64])
            qcols.append(w_qkv[:, 64 * heads[j + 4] : 64 * heads[j + 4] + 64])
        kcol = w_qkv[:, D + 128 * gp : D + 128 * gp + 128]
        vcol = w_qkv[:, D + KV_DIM + 128 * gp : D + KV_DIM + 128 * gp + 128]
        wq = np.ascontiguousarray(np.concatenate(qcols + [kcol, vcol], axis=1).astype(ml_dtypes.bfloat16))
        # w_proj rows in ypair order: pair j = [head j ; head j+4]
        wrows = []
        for j in range(4):
            wrows.append(w_proj[64 * heads[j] : 64 * heads[j] + 64, :])
            wrows.append(w_proj[64 * heads[j + 4] : 64 * heads[j + 4] + 64, :])
        wp = np.ascontiguousarray(np.concatenate(wrows, axis=0).astype(ml_dtypes.bfloat16))
        maps.append(
            {
                "xt": xt[b],
                "wqkv": wq,
                "wproj": wp,
                "costab": cos128,
                "sintab": sin128,
                "swp": swp,
                "umask2": umask2,
                "selb": selb,
                "sel4": sel4,
            }
        )
    return maps


# ------------------------------------------------------------------- builder


def build_nc():
    nc = bacc.Bacc("TRN2", target_bir_lowering=False, debug=False, num_devices=NCORES)
    xt_d = nc.dram_tensor("xt", [D, T], BF16, kind="ExternalInput").ap()
    wq_d = nc.dram_tensor("wqkv", [D, WCOLS], BF16, kind="ExternalInput").ap()
    wp_d = nc.dram_tensor("wproj", [512, D], BF16, kind="ExternalInput").ap()
    cos_d = nc.dram_tensor("costab", [128, T], F32, kind="ExternalInput").ap()
    sin_d = nc.dram_tensor("sintab", [128, T], F32, kind="ExternalInput").ap()
    swp_d = nc.dram_tensor("swp", [128, 128], BF16, kind="ExternalInput").ap()
    um2_d = nc.dram_tensor("umask2", [128, 2, 128], BF16, kind="ExternalInput").ap()
    slb_d = nc.dram_tensor("selb", [128, 128], BF16, kind="ExternalInput").ap()
    s4_d = nc.dram_tensor("sel4", [4, 128, 128], BF16, kind="ExternalInput").ap()
    out_d = nc.dram_tensor("out", [T, D], BF16, kind="ExternalOutput").ap()

    with (
        nc.allow_low_precision(reason="bf16 matmul operands; fp32 psum accumulation"),
        tile.TileContext(nc) as tc,
        ExitStack() as ctx,
    ):
        const = ctx.enter_context(tc.tile_pool(name="const", bufs=1))
        keep = ctx.enter_context(tc.tile_pool(name="keep", bufs=1))
        p_x = ctx.enter_context(tc.tile_pool(name="p_x", bufs=32))
        p_w = ctx.enter_context(tc.tile_pool(name="p_w", bufs=2))
        p_yh = ctx.enter_context(tc.tile_pool(name="p_yh", bufs=4))
        p_pt = ctx.enter_context(tc.tile_pool(name="p_pt", bufs=3))
        ps_a = ctx.enter_context(tc.tile_pool(name="ps_a", bufs=1, space="PSUM"))
        ps_sc = ctx.enter_context(tc.tile_pool(name="ps_sc", bufs=2, space="PSUM"))
        ps_pv = ctx.enter_context(tc.tile_pool(name="ps_pv", bufs=1, space="PSUM"))

        cos_t = const.tile([128, T], F32)
        sin_t = const.tile([128, T], F32)
        swp_t = const.tile([128, 128], BF16)
        um2_t = const.tile([128, 2, 128], BF16)
        slb_t = const.tile([128, 128], BF16)
        s4_t = const.tile([128, 4, 128], BF16)
        wq_sb = [
            keep.tile([128, WCOLS], BF16, tag=f"wq{ci}", name=f"wq{ci}")
            for ci in range(NCT)
        ]
        wp_sb = keep.tile([128, 4, D], BF16, tag="wp", name="wp_sb")

        # persistent per-chunk activations
        qpad = [
            [
                [
                    keep.tile([128, TQ], BF16, tag=f"qp{c}_{jp}_{h}", name=f"qp{c}_{jp}_{h}")
                    for h in range(2)
                ]
                for jp in range(4)
            ]
            for c in range(NT)
        ]
        kp_c = [keep.tile([128, TQ], BF16, tag=f"kp{c}", name=f"kp{c}") for c in range(NT)]
        v_c = [keep.tile([128, TQ], BF16, tag=f"v{c}", name=f"v{c}") for c in range(NT)]
        va_c = [
            [keep.tile([128, 4, 65], BF16, tag=f"va{c}_{g}", name=f"va{c}_{g}") for g in range(2)]
            for c in range(NT)
        ]
        yp_c = [
            [keep.tile([128, TQ], BF16, tag=f"yp{c}_{jj}", name=f"yp{c}_{jj}") for jj in range(4)]
            for c in range(NT)
        ]
        sump_c = [keep.tile([128, TQ], BF16, tag=f"sump{c}", name=f"sump{c}") for c in range(NT)]

        # sump rows 8-127 are never written but are read by the bcast matmul
        # (times zero stationary cols) and the denominator reciprocal -- they
        # must be finite; qpad padding halves must be exact zeros for the
        # K=128 padded scores
        for c in range(NT):
            nc.gpsimd.memset(sump_c[c][:], 1.0)
        for c in range(NT):
            for jp in range(4):
                nc.gpsimd.memset(qpad[c][jp][0][64:128, :], 0.0)
                nc.gpsimd.memset(qpad[c][jp][1][0:64, :], 0.0)

        xts = [[None] * NCT for _ in range(NT)]

        def dma_consts():
            for t_, d_ in [
                (um2_t, um2_d),
                (slb_t, slb_d),
            ]:
                nc.sync.dma_start(t_[:], d_)
            for jj in range(4):
                nc.sync.dma_start(s4_t[:, jj, :], s4_d[jj, :, :])
            for fi in range(4):
                nc.sync.dma_start(wp_sb[:, fi, :], wp_d[128 * fi : 128 * fi + 128, :])

        def dma_xt(c):
            ts = slice(TQ * c, TQ * c + TQ)
            for ci in range(NCT):
                xt_t = p_x.tile([128, TQ], BF16, tag="xt", name=f"xt{c}_{ci}")
                nc.sync.dma_start(xt_t[:], xt_d[128 * ci : 128 * ci + 128, ts])
                xts[c][ci] = xt_t

        def rope(c, ps, dest_q=None, dest_k=None):
            ts = slice(TQ * c, TQ * c + TQ)
            raw = p_w.tile([128, TQ], BF16, tag="raw", name="raw")
            nc.vector.tensor_copy(raw[:], ps[:])
            sw = ps_a.tile([128, TQ], F32, tag="qkv", name="sw")
            nc.tensor.matmul(sw[:], swp_t[:], raw[:], start=True, stop=True)
            t1 = p_w.tile([128, TQ], F32, tag="t1", name="t1")
            t2 = p_w.tile([128, TQ], F32, tag="t2", name="t2")
            nc.vector.tensor_mul(t1[:], raw[:], cos_t[:, ts])
            nc.vector.tensor_mul(t2[:], sw[:], sin_t[:, ts])
            if dest_q is not None:
                nc.vector.tensor_add(dest_q[0][0:64, :], t1[0:64, :], t2[0:64, :])
                nc.vector.tensor_add(dest_q[1][64:128, :], t1[64:128, :], t2[64:128, :])
            else:
                nc.vector.tensor_add(dest_k[:], t1[:], t2[:])

        def qkv_part(c, ocs):
            for oc in ocs:
                ps = ps_a.tile([128, TQ], F32, tag="qkv", name="qkv")
                for ci in range(NCT):
                    nc.tensor.matmul(
                        ps[:],
                        wq_sb[ci][:, 128 * oc : 128 * oc + 128],
                        xts[c][ci][:],
                        start=(ci == 0),
                        stop=(ci == NCT - 1),
                    )
                if oc < 4:
                    rope(c, ps, dest_q=qpad[c][oc])
                elif oc == 4:
                    rope(c, ps, dest_k=kp_c[c])
                else:
                    nc.vector.tensor_copy(v_c[c][:], ps[:])
                    for kloc in range(4):
                        vp = ps_a.tile([128, 128], F32, tag="qkv", name="vp")
                        nc.tensor.matmul(
                            vp[:],
                            v_c[c][:, 128 * kloc : 128 * kloc + 128],
                            slb_t[:],
                            start=True,
                            stop=True,
                        )
                        nc.vector.tensor_copy(va_c[c][0][:, kloc, 0:64], vp[:, 0:64])
                        nc.vector.tensor_copy(va_c[c][1][:, kloc, 0:64], vp[:, 64:128])
                    for g in range(2):
                        nc.gpsimd.memset(va_c[c][g][:, :, 64:65], 1.0)

        def attn(s, jp):
            pv = [
                ps_pv.tile([65, TQ], F32, tag=f"pv{h}", name=f"pv{h}") for h in range(2)
            ]
            nkj = 4 * s + 4

            def emit_sc(kj):
                qcs, kloc = kj // 4, kj % 4
                col0 = max(kj * 128 - s * TQ, 0)
                sc = ps_sc.tile([128, 2, TQ], F32, tag="sc", name="sc")
                for h in range(2):
                    nc.tensor.matmul(
                        sc[:, h, col0:TQ],
                        kp_c[qcs][:, 128 * kloc : 128 * kloc + 128],
                        qpad[s][jp][h][:, col0:TQ],
                        start=True,
                        stop=True,
                    )
                pt = p_pt.tile([128, 2, TQ], BF16, tag="pt", name="pt")
                nc.scalar.activation(
                    pt[:, :, col0:TQ],
                    sc[:, :, col0:TQ],
                    mybir.ActivationFunctionType.Exp,
                    scale=SCALE,
                )
                if kj >= 4 * s:  # diagonal tile: triangular keep-mask, both heads
                    nc.gpsimd.tensor_mul(
                        pt[:, :, col0 : col0 + 128],
                        pt[:, :, col0 : col0 + 128],
                        um2_t[:],
                    )
                return pt, col0

            def emit_pv(kj, pt, col0):
                qcs, kloc = kj // 4, kj % 4
                for h in range(2):
                    nc.tensor.matmul(
                        pv[h][:, col0:TQ],
                        va_c[qcs][h][:, kloc, :],
                        pt[:, h, col0:TQ],
                        start=(kj == 0),
                        stop=(kj == nkj - 1),
                    )

            # 1-deep software pipeline: the PE queue is in-order, so pv(kj)
            # must sit behind sc(kj+1) or the PE stalls on exp(kj) every step
            prev = emit_sc(0)
            for kj in range(1, nkj):
                cur = emit_sc(kj)
                emit_pv(kj - 1, *prev)
                prev = cur
            emit_pv(nkj - 1, *prev)
            nc.vector.tensor_copy(yp_c[s][jp][0:64, :], pv[0][0:64, :])
            dn = p_yh.tile([65, TQ], BF16, tag="dn", name="dn")
            nc.vector.tensor_copy(dn[64:65, :], pv[0][64:65, :])
            nc.sync.dma_start(sump_c[s][2 * jp : 2 * jp + 1, :], dn[64:65, :])
            yh = p_yh.tile([65, TQ], BF16, tag="yh", name="yh")
            nc.vector.tensor_copy(yh[:], pv[1][:])
            nc.sync.dma_start(yp_c[s][jp][64:128, :], yh[0:64, :])
            nc.sync.dma_start(sump_c[s][2 * jp + 1 : 2 * jp + 2, :], yh[64:65, :])

        def norm_recip(c):
            nc.vector.reciprocal(sump_c[c][0:8, :], sump_c[c][0:8, :])

        def norm_bcast(c):
            for jj in range(4):
                bc = ps_a.tile([128, TQ], F32, tag="pj", name="bc")
                nc.tensor.matmul(
                    bc[:], s4_t[:, jj, :], sump_c[c][:], start=True, stop=True
                )
                nc.vector.tensor_mul(yp_c[c][jj][:], yp_c[c][jj][:], bc[:])

        def proj_part(c, tb, wide=False):
            tok0 = c * TQ + tb * 128
            for oc in range(4):
                k = tb * 4 + oc
                if wide and k % 2 == 1:  # epilogue: borrow idle sc banks too
                    pj = ps_sc.tile([128, 2, TQ], F32, tag="sc", name="pj")[:, 0, :]
                else:
                    pj = ps_a.tile(
                        [128, TQ], F32, tag="pj" if k % 2 == 0 else "qkv", name="pj"
                    )
                for jj in range(4):
                    nc.tensor.matmul(
                        pj[:],
                        yp_c[c][jj][:, 128 * tb : 128 * tb + 128],
                        wp_sb[:, jj, TQ * oc : TQ * oc + TQ],
                        start=(jj == 0),
                        stop=(jj == 3),
                    )
                ot = p_w.tile([128, TQ], BF16, tag="ot", name="ot")
                nc.vector.tensor_copy(ot[:], pj[:])
                nc.sync.dma_start(out_d[tok0 : tok0 + 128, TQ * oc : TQ * oc + TQ], ot[:])

        # ---------------- emission: chunk-pipelined schedule
        # qkv-critical loads paired per-ci so the first matmul starts ~2us in
        for ci in range(NCT):
            nc.sync.dma_start(wq_sb[ci][:], wq_d[128 * ci : 128 * ci + 128, :])
            xt_t = p_x.tile([128, TQ], BF16, tag="xt", name=f"xt0_{ci}")
            nc.sync.dma_start(xt_t[:], xt_d[128 * ci : 128 * ci + 128, 0:TQ])
            xts[0][ci] = xt_t
            if ci == 7:
                nc.sync.dma_start(cos_t[:], cos_d)
                nc.sync.dma_start(sin_t[:], sin_d)
                nc.sync.dma_start(swp_t[:], swp_d)
        dma_consts()
        dma_xt(1)
        qkv_part(0, [4, 5, 0, 1])
        oc_slices = [[0, 1], [2, 3], [4], [5]]
        for s in range(NT):
            if s >= 1 and s + 1 < NT:
                dma_xt(s + 1)  # chunk s+1 inputs, consumed by qkv_part this step
            if s >= 1:
                norm_recip(s - 1)  # DVE-early: ready before the bcast matmuls
            for jp in range(4):
                attn(s, jp)
                if s == 0 and jp < 2:
                    qkv_part(0, [jp + 2])
                if s < NT - 1:
                    qkv_part(s + 1, oc_slices[jp])
                if jp == 0 and s >= 1:
                    norm_bcast(s - 1)  # recip ran during attn+qkv above
                if s >= 1:
                    proj_part(s - 1, jp)
        norm_recip(NT - 1)
        norm_bcast(NT - 1)
        for tb in range(4):
            proj_part(NT - 1, tb, wide=True)

    nc.compile()
    return nc


_NC_CACHE = None


def _get_nc():
    global _NC_CACHE
    if _NC_CACHE is None:
        _NC_CACHE = build_nc()
    return _NC_CACHE


def kernel(x, w_qkv, w_proj, _trace=False, _nc=None):
    x = np.asarray(x, np.float32)
    w_qkv = np.asarray(w_qkv, np.float32)
    w_proj = np.asarray(w_proj, np.float32)
    nc = _nc if _nc is not None else _get_nc()
    in_maps = _shard_inputs(x, w_qkv, w_proj)
    res = run_bass_kernel_spmd(nc, in_maps, core_ids=list(range(NCORES)), trace=_trace)
    out = np.zeros((B, T, D), np.float32)
    for i in range(NCORES):
        out[i // 4] += res.results[i]["out"].astype(np.float32)
    if _trace:
        return out, res
    return out


if __name__ == "__main__":
    rng = np.random.default_rng(0)
    x = rng.standard_normal((B, T, D), dtype=np.float32)
    wq = rng.standard_normal((D, D + 2 * KV_DIM), dtype=np.float32) * D**-0.5
    wp = rng.standard_normal((D, D), dtype=np.float32) * D**-0.5
    y = kernel(x, wq, wp)
    print(y.shape, y.dtype)
